# revision 13
# baseline (speedup 1.0000x reference)
"""Trainium2 Bass kernel for the ReActNet-style binary conv building block.

Data-parallel across 8 NeuronCores (8 samples each). v2 design:
- conv1/conv2 as fp8e4 DoubleRow matmuls (2 contraction k-tiles per
  instruction, 2x PE throughput): BP holds sign(x) in {-1,+1} fp8,
  s24 holds sign(y)/2 in {-0.5,+0.5} fp8, weights are sign(W) fp8.
- BN + RPReLU + shortcut-BN fold into ONE per-channel Prelu activation
  per conv (scale/bias/alpha are per-partition vectors), since the
  shortcut-BN scale sinv > 0 commutes through PReLU.
- quant4 rounding via the bf16 cast trick bf16(7.5*x + 199.5) computed
  on the GpSimd engine; clip on DVE (2x bf16); 2x2 sum-pool via
  identity-weight matmuls on the PE.
- Stage-2 output z = Prelu(ps2) + (E2*rc2 + D2) assembled on DVE.
"""

import sys

sys.path.insert(0, "/opt/trn_rl_repo")

import numpy as np
import ml_dtypes

B_PER_CORE = 8
N_CORES = 8
CIN = 256
COUT = 512
H = 28
W = 28
HO = 14
WO = 14
PIX = HO * WO  # 196
NG = 2  # samples per group
NCOL = NG * PIX  # 392 matmul free size

# padded image layout (rows 0..29, cols 0..31); interior at [1:29, 2:30]
PH, PW = 30, 32

_PROGRAM_CACHE = {}


def _build_program():
    if "nc" in _PROGRAM_CACHE:
        return _PROGRAM_CACHE["nc"]

    import concourse.bacc as bacc
    import concourse.tile as tile
    from concourse import mybir

    f32 = mybir.dt.float32
    bf16 = mybir.dt.bfloat16
    fp8 = mybir.dt.float8e4
    Alu = mybir.AluOpType
    Act = mybir.ActivationFunctionType
    DR = mybir.MatmulPerfMode.DoubleRow

    nc = bacc.Bacc(
        "TRN2",
        target_bir_lowering=False,
        debug=False,
        enable_asserts=False,
        num_devices=N_CORES,
    )

    xs_d = nc.dram_tensor("xs", [B_PER_CORE, 2, 128, H * W], f32, kind="ExternalInput")
    w3_d = nc.dram_tensor("w3s", [128, 2 * 9 * 2 * 128], fp8, kind="ExternalInput")
    w1_d = nc.dram_tensor("w1s", [128, 4 * 2 * 128], fp8, kind="ExternalInput")
    dg_d = nc.dram_tensor("dg", [128, 128], bf16, kind="ExternalInput")
    cv_d = nc.dram_tensor("cv", [128, 32], f32, kind="ExternalInput")
    out_d = nc.dram_tensor(
        "out", [B_PER_CORE, 4, 128, PIX], f32, kind="ExternalOutput"
    )

    with tile.TileContext(nc) as tc:
        with (
            tc.tile_pool(name="consts", bufs=1) as cpool,
            tc.tile_pool(name="xin", bufs=4) as xpool,
            tc.tile_pool(name="rq", bufs=2) as rpool,
            tc.tile_pool(name="bpad", bufs=2) as bpool,
            tc.tile_pool(name="rcq", bufs=2) as rcpool,
            tc.tile_pool(name="sq", bufs=2) as spool,
            tc.tile_pool(name="rc2", bufs=2) as rqpool,
            tc.tile_pool(name="pact", bufs=2) as ppool,
            tc.tile_pool(name="yact", bufs=2) as ypool,
            tc.tile_pool(name="r2t", bufs=2) as r2pool,
            tc.tile_pool(name="zp", bufs=3) as zpool,
            tc.tile_pool(name="vv", bufs=3) as vpool,
            tc.tile_pool(name="outs", bufs=3) as opool,
            tc.tile_pool(name="pq", bufs=3, space="PSUM") as pq,
            tc.tile_pool(name="pc1", bufs=2, space="PSUM") as pc1,
            tc.tile_pool(name="pc2", bufs=2, space="PSUM") as pc2,
        ):
            W3S = cpool.tile([128, 2 * 9 * 2 * 128], fp8)
            W1S = cpool.tile([128, 4 * 2 * 128], fp8)
            DG = cpool.tile([128, 128], bf16)
            CV = cpool.tile([128, 32], f32)
            # consts on the Scalar engine's DMA queue so they don't delay
            # the first input loads on the SP queue
            nc.scalar.dma_start(CV[:], cv_d[:])
            nc.scalar.dma_start(DG[:], dg_d[:])
            nc.scalar.dma_start(W3S[:], w3_d[:])
            nc.scalar.dma_start(W1S[:], w1_d[:])

            W3v = W3S[:].rearrange("p (j t c m) -> p j t c m", j=2, t=9, c=2)
            W1v = W1S[:].rearrange("p (j c m) -> p j c m", j=4, c=2)

            def cvec(col):
                return CV[:, col : col + 1]

            for g in range(4):
                BP = bpool.tile([128, 2, NG, PH, PW], fp8, tag="bpad")
                nc.gpsimd.memset(BP[:, :, :, 0, :], 0.0)
                nc.gpsimd.memset(BP[:, :, :, 1:29, 1], 0.0)
                RCg = rcpool.tile([128, 2, NG, H * W], bf16, tag="rc")
                for si in range(NG):
                    s = NG * g + si
                    X = xpool.tile([128, 2, H * W], f32, tag="x")
                    nc.sync.dma_start(X[:], xs_d[s].rearrange("c p hw -> p c hw"))
                    Xv = X[:].rearrange("p c (h w) -> p c h w", h=H, w=W)
                    # sign(x) in {-1,+1} fp8 into padded tile
                    nc.scalar.activation(BP[:, :, si, 1:29, 2:30], Xv, Act.Sign)
                    # R = bf16(7.5*x + 199.5): rounds to int grid (r+192)
                    R = rpool.tile([128, 2, H * W], bf16, tag="r")
                    nc.gpsimd.tensor_scalar(
                        R[:], X[:], 7.5, 199.5, Alu.mult, Alu.add
                    )
                    nc.vector.tensor_scalar(
                        RCg[:, :, si, :], R[:], 207.0, 192.0, Alu.min, Alu.max
                    )

                # 2x2 sum-pool into PSUM via identity matmuls
                RCv = RCg[:].rearrange(
                    "p c s (y a x b) -> p c s y a x b", y=HO, a=2, x=WO, b=2
                )
                Q2 = [
                    pq.tile([128, 512], f32, tag="pq", name=f"q2_{g}_{j}")
                    for j in range(2)
                ]
                for j in range(2):
                    qout = Q2[j][:, :NCOL].rearrange(
                        "p (s y x) -> p s y x", s=NG, y=HO
                    )
                    for t in range(4):
                        ph, pw = t >> 1, t & 1
                        nc.tensor.matmul(
                            qout,
                            DG[:],
                            RCv[:, j, :, :, ph, :, pw],
                            start=(t == 0),
                            stop=(t == 3),
                        )

                s24g = spool.tile([128, 2, NCOL], fp8, tag="s24")
                rc24 = rqpool.tile([128, 2, NCOL], bf16, tag="rc24")
                for j in range(2):
                    ps1 = pc1.tile([128, 512], f32, tag="ps1")
                    # conv1: 9 DoubleRow matmuls (c-halves paired per tap)
                    # per sample (ifmap AP limited to 3 free dims)
                    for si in range(NG):
                        om = ps1[:, si * PIX : (si + 1) * PIX].rearrange(
                            "p (y x) -> p y x", y=HO
                        )
                        for t in range(9):
                            kh, kw = t // 3, t % 3
                            rhs = BP[
                                :, :, si, kh : kh + 28, kw + 1 : kw + 29
                            ].rearrange(
                                "p c (y a) (x b) -> p c y a x b", a=2, b=2
                            )[:, :, :, 0, :, 0]
                            nc.tensor.matmul(
                                om,
                                W3v[:, j, t],
                                rhs,
                                start=(t == 0),
                                stop=(t == 8),
                                perf_mode=DR,
                            )
                    # P = Prelu(sA1*ps1 + bA1, alpha=beta1)
                    P = ppool.tile([128, NCOL], f32, tag="P")
                    nc.scalar.activation(
                        P[:], ps1[:, :NCOL], Act.Prelu,
                        bias=cvec(2 + j), scale=cvec(0 + j), alpha=cvec(4 + j),
                    )
                    # y = E1*Q2 + P   (D1tot folded downstream)
                    Y = ypool.tile([128, NCOL], f32, tag="y")
                    nc.vector.scalar_tensor_tensor(
                        Y[:], Q2[j][:, :NCOL], cvec(6 + j), P[:],
                        Alu.mult, Alu.add,
                    )
                    R2 = r2pool.tile([128, NCOL], bf16, tag="r2")
                    nc.gpsimd.tensor_scalar(
                        R2[:], Y[:], 7.5, cvec(8 + j), Alu.mult, Alu.add
                    )
                    nc.vector.tensor_scalar(
                        rc24[:, j, :], R2[:], 207.0, 192.0, Alu.min, Alu.max
                    )
                    # s2/2 in {-0.5,+0.5} fp8: (y >= -D1tot) - 0.5
                    nc.vector.tensor_scalar(
                        s24g[:, j, :], Y[:], cvec(10 + j), 0.5,
                        Alu.is_ge, Alu.subtract,
                    )

                # stage 2
                OS = opool.tile([128, NG, 4, PIX], f32, tag="o")
                for jj in range(4):
                    ps2 = pc2.tile([128, 512], f32, tag="ps2")
                    nc.tensor.matmul(
                        ps2[:, :NCOL], W1v[:, jj], s24g[:],
                        start=True, stop=True, perf_mode=DR,
                    )
                    ZP = zpool.tile([128, NCOL], f32, tag="zp")
                    nc.scalar.activation(
                        ZP[:], ps2[:, :NCOL], Act.Prelu,
                        bias=cvec(16 + jj), scale=cvec(12 + jj),
                        alpha=cvec(20 + jj),
                    )
                    # V = E2*rc2 + D2 (bf16, 2x DVE)
                    V = vpool.tile([128, NCOL], bf16, tag="v")
                    nc.vector.tensor_scalar(
                        V[:], rc24[:, jj % 2, :], cvec(24 + jj), cvec(28 + jj),
                        Alu.mult, Alu.add,
                    )
                    nc.vector.tensor_tensor(
                        OS[:, :, jj, :], V[:].rearrange("p (s x) -> p s x", s=NG),
                        ZP[:].rearrange("p (s x) -> p s x", s=NG), Alu.add,
                    )
                # one output DMA per group on the GpSimd (software DGE)
                # queue, so output never head-of-line-blocks input prefetch
                nc.gpsimd.dma_start(
                    out_d[NG * g : NG * g + 2].rearrange("s j p x -> p s j x"),
                    OS[:],
                )

    nc.compile()
    _PROGRAM_CACHE["nc"] = nc
    return nc


def _prep_consts(
    w3, w1,
    bn1_m, bn1_v, bn1_w, bn1_b,
    bn2_m, bn2_v, bn2_w, bn2_b,
    sbn1_m, sbn1_v, sbn1_w, sbn1_b,
    sbn2_m, sbn2_v, sbn2_w, sbn2_b,
    rp1_gamma, rp1_beta, rp1_zeta,
    rp2_gamma, rp2_beta, rp2_zeta,
):
    f = np.float32
    bf = ml_dtypes.bfloat16
    f8 = ml_dtypes.float8_e4m3
    eps = f(1e-5)
    w3 = w3.astype(f)
    w1 = w1.astype(f)

    inv1 = bn1_w / np.sqrt(bn1_v + eps)
    shift1 = bn1_b - bn1_m * inv1
    alpha3 = np.mean(np.abs(w3), axis=(1, 2, 3))
    s3 = np.where(w3 >= 0, f(1.0), f(-1.0))
    sinv1 = sbn1_w / np.sqrt(sbn1_v + eps)
    sshift1 = sbn1_b - sbn1_m * sinv1
    A1 = alpha3 * inv1
    base1 = shift1 - rp1_gamma
    sA1 = A1 * sinv1
    bA1 = base1 * sinv1
    E1 = sinv1 / f(30.0)
    D1tot = rp1_zeta * sinv1 + sshift1 - sinv1 - f(768.0) * E1
    r2bias = f(199.5) + f(7.5) * D1tot

    inv2 = bn2_w / np.sqrt(bn2_v + eps)
    shift2 = bn2_b - bn2_m * inv2
    alpha1 = np.mean(np.abs(w1), axis=(1, 2, 3))
    s1 = np.where(w1 >= 0, f(1.0), f(-1.0))
    sinv2 = sbn2_w / np.sqrt(sbn2_v + eps)
    sshift2 = sbn2_b - sbn2_m * sinv2
    A2 = alpha1 * inv2
    base2 = shift2 - rp2_gamma
    sA2 = f(2.0) * A2 * sinv2
    bA2 = base2 * sinv2
    E2 = f(2.0 / 15.0) * sinv2
    D2tot = rp2_zeta * sinv2 + sshift2 - sinv2 - f(192.0) * E2

    cv = np.zeros((128, 32), dtype=f)
    for j in range(2):
        sl = slice(j * 128, (j + 1) * 128)
        cv[:, 0 + j] = sA1[sl]
        cv[:, 2 + j] = bA1[sl]
        cv[:, 4 + j] = rp1_beta[sl]
        cv[:, 6 + j] = E1[sl]
        cv[:, 8 + j] = r2bias[sl]
        cv[:, 10 + j] = -D1tot[sl]
    for jj in range(4):
        sl = slice(jj * 128, (jj + 1) * 128)
        cv[:, 12 + jj] = sA2[sl]
        cv[:, 16 + jj] = bA2[sl]
        cv[:, 20 + jj] = rp2_beta[sl]
        cv[:, 24 + jj] = E2[sl]
        cv[:, 28 + jj] = D2tot[sl]

    # conv1 lhsT [k, (j t c m)] fp8; o = j*128+m, i = c*128+k, t = kh*3+kw
    w3l = (
        s3.reshape(2, 128, 2, 128, 3, 3)
        .transpose(3, 0, 4, 5, 2, 1)  # [k, j, kh, kw, c, m]
        .reshape(128, 2 * 9 * 2 * 128)
        .astype(f8)
    )
    # conv2 lhsT [k, (jj c m)] fp8; o = jj*128+m, i = c*128+k
    w1l = (
        s1.reshape(4, 128, 2, 128)
        .transpose(3, 0, 2, 1)  # [k, jj, c, m]
        .reshape(128, 4 * 2 * 128)
        .astype(f8)
    )
    dg = np.eye(128, dtype=bf)
    return w3l, w1l, dg, cv


def run(inputs, trace=False):
    from concourse import bass_utils

    nc = _build_program()
    x = np.asarray(inputs["x"], dtype=np.float32)
    w3l, w1l, dg, cv = _prep_consts(
        **{k: np.asarray(v, np.float32) for k, v in inputs.items() if k != "x"}
    )

    in_maps = []
    for core in range(N_CORES):
        xs = (
            x[core * B_PER_CORE : (core + 1) * B_PER_CORE]
            .reshape(B_PER_CORE, 2, 128, H * W)
            .copy()
        )
        in_maps.append({"xs": xs, "w3s": w3l, "w1s": w1l, "dg": dg, "cv": cv})

    res = bass_utils.run_bass_kernel_spmd(
        nc, in_maps, core_ids=list(range(N_CORES)), trace=trace
    )
    outs = [
        res.results[c]["out"].reshape(B_PER_CORE, COUT, HO, WO)
        for c in range(N_CORES)
    ]
    full = np.concatenate(outs, axis=0)
    return full, res


def kernel(**inputs):
    out, _ = run(inputs, trace=False)
    return out


# revision 17
# speedup vs baseline: 1.1019x; 1.1019x over previous
"""Trainium2 Bass kernel for the ReActNet-style binary conv building block.

Data-parallel across 8 NeuronCores (8 samples each). v2 design:
- conv1/conv2 as fp8e4 DoubleRow matmuls (2 contraction k-tiles per
  instruction, 2x PE throughput): BP holds sign(x) in {-1,+1} fp8,
  s24 holds sign(y)/2 in {-0.5,+0.5} fp8, weights are sign(W) fp8.
- BN + RPReLU + shortcut-BN fold into ONE per-channel Prelu activation
  per conv (scale/bias/alpha are per-partition vectors), since the
  shortcut-BN scale sinv > 0 commutes through PReLU.
- quant4 rounding via the bf16 cast trick bf16(7.5*x + 199.5) computed
  on the GpSimd engine; clip on DVE (2x bf16); 2x2 sum-pool via
  identity-weight matmuls on the PE.
- Stage-2 output z = Prelu(ps2) + (E2*rc2 + D2) assembled on DVE.
"""

import sys

sys.path.insert(0, "/opt/trn_rl_repo")

import numpy as np
import ml_dtypes

B_PER_CORE = 8
N_CORES = 8
CIN = 256
COUT = 512
H = 28
W = 28
HO = 14
WO = 14
PIX = HO * WO  # 196
NG = 2  # samples per group
NCOL = NG * PIX  # 392 matmul free size

# padded image layout (rows 0..29, cols 0..31); interior at [1:29, 2:30]
PH, PW = 30, 32

_PROGRAM_CACHE = {}


def _build_program():
    if "nc" in _PROGRAM_CACHE:
        return _PROGRAM_CACHE["nc"]

    import concourse.bacc as bacc
    import concourse.tile as tile
    from concourse import mybir

    f32 = mybir.dt.float32
    bf16 = mybir.dt.bfloat16
    fp8 = mybir.dt.float8e4
    Alu = mybir.AluOpType
    Act = mybir.ActivationFunctionType
    DR = mybir.MatmulPerfMode.DoubleRow

    nc = bacc.Bacc(
        "TRN2",
        target_bir_lowering=False,
        debug=False,
        enable_asserts=False,
        num_devices=N_CORES,
    )

    xs_d = nc.dram_tensor("xs", [B_PER_CORE, 2, 128, H * W], f32, kind="ExternalInput")
    w3_d = nc.dram_tensor("w3s", [128, 2 * 9 * 2 * 128], fp8, kind="ExternalInput")
    w1_d = nc.dram_tensor("w1s", [128, 4 * 2 * 128], fp8, kind="ExternalInput")
    dg_d = nc.dram_tensor("dg", [128, 128], bf16, kind="ExternalInput")
    cv_d = nc.dram_tensor("cv", [128, 32], f32, kind="ExternalInput")
    out_d = nc.dram_tensor(
        "out", [B_PER_CORE, 4, 128, PIX], f32, kind="ExternalOutput"
    )

    with tile.TileContext(nc) as tc:
        with (
            tc.tile_pool(name="consts", bufs=1) as cpool,
            tc.tile_pool(name="xin", bufs=8) as xpool,
            tc.tile_pool(name="rq", bufs=2) as rpool,
            tc.tile_pool(name="bpad", bufs=2) as bpool,
            tc.tile_pool(name="rcq", bufs=2) as rcpool,
            tc.tile_pool(name="sq", bufs=2) as spool,
            tc.tile_pool(name="rc2", bufs=2) as rqpool,
            tc.tile_pool(name="pact", bufs=2) as ppool,
            tc.tile_pool(name="yact", bufs=2) as ypool,
            tc.tile_pool(name="r2t", bufs=2) as r2pool,
            tc.tile_pool(name="zp", bufs=3) as zpool,
            tc.tile_pool(name="vv", bufs=3) as vpool,
            tc.tile_pool(name="outs", bufs=3) as opool,
            tc.tile_pool(name="pq", bufs=3, space="PSUM") as pq,
            tc.tile_pool(name="pc1", bufs=2, space="PSUM") as pc1,
            tc.tile_pool(name="pc2", bufs=2, space="PSUM") as pc2,
        ):
            W3S = cpool.tile([128, 2 * 9 * 2 * 128], fp8)
            W1S = cpool.tile([128, 4 * 2 * 128], fp8)
            DG = cpool.tile([128, 128], bf16)
            CV = cpool.tile([128, 32], f32)
            # consts on the Scalar engine's DMA queue so they don't delay
            # the first input loads on the SP queue
            nc.scalar.dma_start(CV[:], cv_d[:])
            nc.scalar.dma_start(DG[:], dg_d[:])
            nc.scalar.dma_start(W3S[:], w3_d[:])
            nc.scalar.dma_start(W1S[:], w1_d[:])

            W3v = W3S[:].rearrange("p (j t c m) -> p j t c m", j=2, t=9, c=2)
            W1v = W1S[:].rearrange("p (j c m) -> p j c m", j=4, c=2)

            def cvec(col):
                return CV[:, col : col + 1]

            # issue ALL input loads up front on the SP queue: they have no
            # wait conditions, so the queue streams them back-to-back and
            # later output DMAs can't head-of-line-block input prefetch
            Xs = []
            for s in range(B_PER_CORE):
                X = xpool.tile([128, 2, H * W], f32, tag="x", name=f"x_{s}")
                nc.sync.dma_start(X[:], xs_d[s].rearrange("c p hw -> p c hw"))
                Xs.append(X)

            for g in range(4):
                BP = bpool.tile([128, 2, NG, PH, PW], fp8, tag="bpad")
                nc.gpsimd.memset(BP[:, :, :, 0, :], 0.0)
                nc.gpsimd.memset(BP[:, :, :, 1:29, 1], 0.0)
                RCg = rcpool.tile([128, 2, NG, H * W], bf16, tag="rc")
                for si in range(NG):
                    s = NG * g + si
                    X = Xs[s]
                    Xv = X[:].rearrange("p c (h w) -> p c h w", h=H, w=W)
                    # sign(x) in {-1,+1} fp8 into padded tile
                    nc.scalar.activation(BP[:, :, si, 1:29, 2:30], Xv, Act.Sign)
                    # R = bf16(7.5*x + 199.5): rounds to int grid (r+192)
                    R = rpool.tile([128, 2, H * W], bf16, tag="r")
                    nc.gpsimd.tensor_scalar(
                        R[:], X[:], 7.5, 199.5, Alu.mult, Alu.add
                    )
                    nc.vector.tensor_scalar(
                        RCg[:, :, si, :], R[:], 207.0, 192.0, Alu.min, Alu.max
                    )

                # 2x2 sum-pool into PSUM via identity matmuls
                RCv = RCg[:].rearrange(
                    "p c s (y a x b) -> p c s y a x b", y=HO, a=2, x=WO, b=2
                )
                Q2 = [
                    pq.tile([128, 512], f32, tag="pq", name=f"q2_{g}_{j}")
                    for j in range(2)
                ]
                for j in range(2):
                    qout = Q2[j][:, :NCOL].rearrange(
                        "p (s y x) -> p s y x", s=NG, y=HO
                    )
                    for t in range(4):
                        ph, pw = t >> 1, t & 1
                        nc.tensor.matmul(
                            qout,
                            DG[:],
                            RCv[:, j, :, :, ph, :, pw],
                            start=(t == 0),
                            stop=(t == 3),
                        )

                s24g = spool.tile([128, 2, NCOL], fp8, tag="s24")
                rc24 = rqpool.tile([128, 2, NCOL], bf16, tag="rc24")
                for j in range(2):
                    ps1 = pc1.tile([128, 512], f32, tag="ps1")
                    # conv1: 9 DoubleRow matmuls (c-halves paired per tap)
                    # per sample (ifmap AP limited to 3 free dims)
                    for si in range(NG):
                        om = ps1[:, si * PIX : (si + 1) * PIX].rearrange(
                            "p (y x) -> p y x", y=HO
                        )
                        for t in range(9):
                            kh, kw = t // 3, t % 3
                            rhs = BP[
                                :, :, si, kh : kh + 28, kw + 1 : kw + 29
                            ].rearrange(
                                "p c (y a) (x b) -> p c y a x b", a=2, b=2
                            )[:, :, :, 0, :, 0]
                            nc.tensor.matmul(
                                om,
                                W3v[:, j, t],
                                rhs,
                                start=(t == 0),
                                stop=(t == 8),
                                perf_mode=DR,
                            )
                    # P = Prelu(sA1*ps1 + bA1, alpha=beta1)
                    P = ppool.tile([128, NCOL], f32, tag="P")
                    nc.scalar.activation(
                        P[:], ps1[:, :NCOL], Act.Prelu,
                        bias=cvec(2 + j), scale=cvec(0 + j), alpha=cvec(4 + j),
                    )
                    # y = E1*Q2 + P   (D1tot folded downstream)
                    Y = ypool.tile([128, NCOL], f32, tag="y")
                    nc.vector.scalar_tensor_tensor(
                        Y[:], Q2[j][:, :NCOL], cvec(6 + j), P[:],
                        Alu.mult, Alu.add,
                    )
                    R2 = r2pool.tile([128, NCOL], bf16, tag="r2")
                    nc.gpsimd.tensor_scalar(
                        R2[:], Y[:], 7.5, cvec(8 + j), Alu.mult, Alu.add
                    )
                    nc.vector.tensor_scalar(
                        rc24[:, j, :], R2[:], 207.0, 192.0, Alu.min, Alu.max
                    )
                    # s2/2 in {-0.5,+0.5} fp8: (y >= -D1tot) - 0.5
                    nc.vector.tensor_scalar(
                        s24g[:, j, :], Y[:], cvec(10 + j), 0.5,
                        Alu.is_ge, Alu.subtract,
                    )

                # stage 2
                OS = opool.tile([128, NG, 4, PIX], f32, tag="o")
                for jj in range(4):
                    ps2 = pc2.tile([128, 512], f32, tag="ps2")
                    nc.tensor.matmul(
                        ps2[:, :NCOL], W1v[:, jj], s24g[:],
                        start=True, stop=True, perf_mode=DR,
                    )
                    ZP = zpool.tile([128, NCOL], f32, tag="zp")
                    nc.scalar.activation(
                        ZP[:], ps2[:, :NCOL], Act.Prelu,
                        bias=cvec(16 + jj), scale=cvec(12 + jj),
                        alpha=cvec(20 + jj),
                    )
                    # V = E2*rc2 + D2 (bf16, 2x DVE)
                    V = vpool.tile([128, NCOL], bf16, tag="v")
                    nc.vector.tensor_scalar(
                        V[:], rc24[:, jj % 2, :], cvec(24 + jj), cvec(28 + jj),
                        Alu.mult, Alu.add,
                    )
                    nc.vector.tensor_tensor(
                        OS[:, :, jj, :], V[:].rearrange("p (s x) -> p s x", s=NG),
                        ZP[:].rearrange("p (s x) -> p s x", s=NG), Alu.add,
                    )
                # output DMAs sit on the SP queue behind the (waitless)
                # input loads, so they can't stall input prefetch
                nc.sync.dma_start(
                    out_d[NG * g : NG * g + 2].rearrange("s j p x -> p s j x"),
                    OS[:],
                )

    nc.compile()
    _PROGRAM_CACHE["nc"] = nc
    return nc


def _prep_consts(
    w3, w1,
    bn1_m, bn1_v, bn1_w, bn1_b,
    bn2_m, bn2_v, bn2_w, bn2_b,
    sbn1_m, sbn1_v, sbn1_w, sbn1_b,
    sbn2_m, sbn2_v, sbn2_w, sbn2_b,
    rp1_gamma, rp1_beta, rp1_zeta,
    rp2_gamma, rp2_beta, rp2_zeta,
):
    f = np.float32
    bf = ml_dtypes.bfloat16
    f8 = ml_dtypes.float8_e4m3
    eps = f(1e-5)
    w3 = w3.astype(f)
    w1 = w1.astype(f)

    inv1 = bn1_w / np.sqrt(bn1_v + eps)
    shift1 = bn1_b - bn1_m * inv1
    alpha3 = np.mean(np.abs(w3), axis=(1, 2, 3))
    s3 = np.where(w3 >= 0, f(1.0), f(-1.0))
    sinv1 = sbn1_w / np.sqrt(sbn1_v + eps)
    sshift1 = sbn1_b - sbn1_m * sinv1
    A1 = alpha3 * inv1
    base1 = shift1 - rp1_gamma
    sA1 = A1 * sinv1
    bA1 = base1 * sinv1
    E1 = sinv1 / f(30.0)
    D1tot = rp1_zeta * sinv1 + sshift1 - sinv1 - f(768.0) * E1
    r2bias = f(199.5) + f(7.5) * D1tot

    inv2 = bn2_w / np.sqrt(bn2_v + eps)
    shift2 = bn2_b - bn2_m * inv2
    alpha1 = np.mean(np.abs(w1), axis=(1, 2, 3))
    s1 = np.where(w1 >= 0, f(1.0), f(-1.0))
    sinv2 = sbn2_w / np.sqrt(sbn2_v + eps)
    sshift2 = sbn2_b - sbn2_m * sinv2
    A2 = alpha1 * inv2
    base2 = shift2 - rp2_gamma
    sA2 = f(2.0) * A2 * sinv2
    bA2 = base2 * sinv2
    E2 = f(2.0 / 15.0) * sinv2
    D2tot = rp2_zeta * sinv2 + sshift2 - sinv2 - f(192.0) * E2

    cv = np.zeros((128, 32), dtype=f)
    for j in range(2):
        sl = slice(j * 128, (j + 1) * 128)
        cv[:, 0 + j] = sA1[sl]
        cv[:, 2 + j] = bA1[sl]
        cv[:, 4 + j] = rp1_beta[sl]
        cv[:, 6 + j] = E1[sl]
        cv[:, 8 + j] = r2bias[sl]
        cv[:, 10 + j] = -D1tot[sl]
    for jj in range(4):
        sl = slice(jj * 128, (jj + 1) * 128)
        cv[:, 12 + jj] = sA2[sl]
        cv[:, 16 + jj] = bA2[sl]
        cv[:, 20 + jj] = rp2_beta[sl]
        cv[:, 24 + jj] = E2[sl]
        cv[:, 28 + jj] = D2tot[sl]

    # conv1 lhsT [k, (j t c m)] fp8; o = j*128+m, i = c*128+k, t = kh*3+kw
    w3l = (
        s3.reshape(2, 128, 2, 128, 3, 3)
        .transpose(3, 0, 4, 5, 2, 1)  # [k, j, kh, kw, c, m]
        .reshape(128, 2 * 9 * 2 * 128)
        .astype(f8)
    )
    # conv2 lhsT [k, (jj c m)] fp8; o = jj*128+m, i = c*128+k
    w1l = (
        s1.reshape(4, 128, 2, 128)
        .transpose(3, 0, 2, 1)  # [k, jj, c, m]
        .reshape(128, 4 * 2 * 128)
        .astype(f8)
    )
    dg = np.eye(128, dtype=bf)
    return w3l, w1l, dg, cv


def run(inputs, trace=False):
    from concourse import bass_utils

    nc = _build_program()
    x = np.asarray(inputs["x"], dtype=np.float32)
    w3l, w1l, dg, cv = _prep_consts(
        **{k: np.asarray(v, np.float32) for k, v in inputs.items() if k != "x"}
    )

    in_maps = []
    for core in range(N_CORES):
        xs = (
            x[core * B_PER_CORE : (core + 1) * B_PER_CORE]
            .reshape(B_PER_CORE, 2, 128, H * W)
            .copy()
        )
        in_maps.append({"xs": xs, "w3s": w3l, "w1s": w1l, "dg": dg, "cv": cv})

    res = bass_utils.run_bass_kernel_spmd(
        nc, in_maps, core_ids=list(range(N_CORES)), trace=trace
    )
    outs = [
        res.results[c]["out"].reshape(B_PER_CORE, COUT, HO, WO)
        for c in range(N_CORES)
    ]
    full = np.concatenate(outs, axis=0)
    return full, res


def kernel(**inputs):
    out, _ = run(inputs, trace=False)
    return out
